# revision 12
# baseline (speedup 1.0000x reference)
"""Trainium2 Bass kernel for GroupNorm + spatial self-attention + residual (v2).

Reference computation (B=1, C=512, H=W=64, 8 heads x 64 dim, GN groups=32):
    x = GroupNorm(hidden_states) -> tokens [N=4096, C]
    q,k,v = x @ {wq,wk,wv}.T  (per-head slices of inner=512)
    out = softmax(q k^T / 8) v   per head
    y = concat_heads(out) @ wo.T + bo + hidden_states

Distribution: head-parallel attention (core h owns head h, reads full x), then
a bf16 AllToAll token-shards the unnormalized attention output; core j computes
the output projection + residual for tokens [512j, 512j+512).

Key structural points vs v1:
  - GroupNorm affine is folded into the QKV weights: the host pre-multiplies
    gamma into wq/wk/wv; the device multiplies per-channel rstd into the
    weights after stats, and the (beta - mean*s) bias becomes per-head bias
    vectors applied at the q/k eviction (ACT bias) and folded into v itself
    (v' = v + cv, via one broadcast add at the v eviction).  No x_norm pass.
  - Ingest pipeline: x is DMA'd in [128,1024] chunks; ScalarE converts each
    chunk to bf16 while VectorE runs bn_stats on the f32 chunk.  Group stats
    via one accumulated block-diagonal matmul; per-channel (mean, rstd) via a
    second tiny expansion matmul.  Zero DRAM round trips in the stats path.
  - Scores matmuls are row-tiled: kT2 [128, 2048] holds keys 0:2048 in
    partitions 0:63 and keys 2048:4096 in partitions 64:127; qT2 [128, N] is
    the q duplicated into both halves (via duplicated weight columns).  Two
    concurrent K=64 matmuls fill the whole PE array.
  - The two score tiles land in one [128, 1024] PSUM tile (2 banks), so exp
    is one big ACT or DVE pass per step (engines alternate by a tuned ratio).
  - The a2a payload is fp8e4m3 (PV rows as-is; the denominator row scaled by
    1/16 to fit e4m3 range via a per-partition scale AP on the eviction).  The
    receiver casts back to bf16 during the SWDGE DMAs, reconstructs 1/den for
    all 8 heads with one broadcast-DMA + Ln + Exp (the 16x un-scale rides the
    Exp bias), and normalizes with a single bf16 DVE multiply.
  - All ScalarE activations are pinned to the natural_log_exp_and_others
    table set at build time: 2 table loads total instead of 38 (the original
    per-jq Ln/Exp eviction thrashed sets at ~5us per switch).
"""

import sys

sys.path.insert(0, "/opt/trn_rl_repo")

import numpy as np

import concourse.bacc as bacc
import concourse.tile as tile
from concourse import mybir
from concourse.bass_utils import run_bass_kernel_spmd

# Pin all ScalarE activations to the one table set that covers every function
# this kernel uses (ln, exp, identity, copy, ...) so the ACT engine performs a
# single table load instead of switching sets between Exp and Ln call sites.
# Applied only for the duration of the build (see _build).
_PIN_SET = "natural_log_exp_and_others"


class _pin_act_tables:
    def __enter__(self):
        self._orig = bacc.get_activation_tables

        def pinned(arch):
            tabs = self._orig(arch)
            if _PIN_SET in tabs:
                shared = tabs[_PIN_SET]
                tabs = {
                    name: (fns if name == _PIN_SET else fns - shared)
                    for name, fns in tabs.items()
                }
            return tabs

        bacc.get_activation_tables = pinned

    def __exit__(self, *a):
        bacc.get_activation_tables = self._orig

C = 512
N = 4096
HEADS = 8
D = 64
GROUPS = 32
CPG = C // GROUPS  # 16 channels per group
EPS = 1e-5
SCALE = D ** -0.5
NCORE = 8
NT = N // NCORE  # 512 tokens per core for the output projection
TQ = 512  # query-chunk per attention jq block (= a2a slot size)
NTQ = N // TQ  # 8
NTK = N // 128  # 32 key blocks of 128
NPAIR = NTK // 2  # 16 row-tiled key-block pairs per jq
CT = C // 128  # 4 channel tiles
VSTRIDE = 80  # fp8 vaug slot stride (bytes %16==0 for DoubleRow ldweights)
NCHUNK = 4  # 1024-col ingest chunks per channel tile

f32 = mybir.dt.float32
bf16 = mybir.dt.bfloat16
f8 = mybir.dt.float8e4
DEN_SCALE = 1.0 / 16.0  # fit den (~3e3..6e3) into fp8e4m3 range for the a2a
LN_DEN_SCALE = float(np.log(DEN_SCALE))
AF = mybir.ActivationFunctionType
ALU = mybir.AluOpType

_nc_cache = {}

# exp(SCALE*x) ~= ((x*EC0 + EC1)^2 + 0.5)^16 -- fused VectorE pass used to
# split softmax exp work between ScalarE and VectorE (see v1 notes).
EC0 = SCALE / float(np.sqrt(512.0))
EC1 = float(np.sqrt(0.5))
# per-jq engine pattern for the 16 pair-steps: True -> VectorE EXP16
EXP_DVE_PAT = tuple((s % 2) == 1 for s in range(16))  # 8 DVE / 8 ACT
SKEW = 2


def _register_exp16():
    from concourse import dve_ops as dops
    from concourse.dve_spec import Spec, Src0, C0, C1, sq

    for op in dops.OPS:
        if op.name == "EXP16_ANT":
            return op
    t = sq(Src0 * C0 + C1) + C2_LEAF
    body = sq(sq(sq(sq(t))))
    spec = Spec(
        body=body,
        reference=lambda in0, in1, s0, s1, imm2: ((in0 * s0 + s1) ** 2 + imm2)
        ** 16,
    )
    op = dops.DveOp("EXP16_ANT", spec, subdim=False, uops_sha={})
    dops.OPS.append(op)
    dops.CUSTOM_DVE_SPECS[op.name] = op.spec
    dops._SUB_OPCODE_FOR_NAME[op.name] = dops._CUSTOM_DVE_ROW_BASE + len(dops.OPS) - 1
    from concourse.dve_uop import DveOpSpec
    from concourse.dve_spec import lower as dve_lower

    for ver in ("v3", "v4"):
        try:
            uops = dve_lower(spec, ver=ver)
            sha = DveOpSpec(
                name=op.name,
                opcode=dops.get_dve_sub_opcode(op.name),
                uops=uops,
                rd1_en=False,
            ).sha(ver)
            op.uops_sha[ver] = sha
        except Exception:
            pass
    return op


from concourse.dve_spec import C2 as C2_LEAF  # noqa: E402

EXP16 = _register_exp16()


def _attention_jq(nc, tc, jq, ps_s, ps_o, pp, kT2, qT2, vaug, a2a_in, sc65):
    ops = ps_o.tile([D + 1, TQ], f32, name="ops", tag="ops", bufs=2)
    p_tiles = {}

    def mm_scores(s):
        sps = ps_s.tile([128, 1024], f32, name="sps", tag="sps", bufs=3)
        nc.tensor.matmul(
            sps[:, 0:512],
            kT2[0:D, s * 128 : (s + 1) * 128],
            qT2[0:D, jq * TQ : (jq + 1) * TQ],
            start=True,
            stop=True,
        )
        nc.tensor.matmul(
            sps[:, 512:1024],
            kT2[D:128, s * 128 : (s + 1) * 128],
            qT2[D:128, jq * TQ : (jq + 1) * TQ],
            start=True,
            stop=True,
        )
        p = pp.tile([128, 1024], bf16, name="p", tag="p", bufs=5)
        if EXP_DVE_PAT[s]:
            nc.vector._custom_dve(
                EXP16, out=p[:, :], in0=sps[:, :], s0=EC0, s1=EC1, imm2=0.5
            )
        else:
            nc.scalar.activation(p[:, :], sps[:, :], AF.Exp, scale=SCALE)
        p_tiles[s] = p

    def mm_pv(s):
        p = p_tiles.pop(s)
        nc.tensor.matmul(
            ops[:, :],
            vaug[:, 2 * s * VSTRIDE : 2 * s * VSTRIDE + D + 1],
            p[:, 0:512],
            start=(s == 0),
            stop=False,
        )
        nc.tensor.matmul(
            ops[:, :],
            vaug[:, (2 * s + 1) * VSTRIDE : (2 * s + 1) * VSTRIDE + D + 1],
            p[:, 512:1024],
            start=False,
            stop=(s == NPAIR - 1),
        )

    for s in range(SKEW):
        mm_scores(s)
    for s in range(SKEW, NPAIR):
        mm_scores(s)
        mm_pv(s - SKEW)
    for s in range(NPAIR - SKEW, NPAIR):
        mm_pv(s)

    # evict unnormalized PV rows + scaled raw denominator row as fp8
    o_sb = pp.tile([D + 1, TQ], f8, name="o_sb", tag="o_sb", bufs=2)
    nc.scalar.activation(o_sb[:, :], ops[:, :], AF.Identity, scale=sc65[:, :])
    nc.sync.dma_start(a2a_in[jq, :, :], o_sb[:, :])


def _build(pre_loop_k=None, attn_loop_k=None, post_loop_k=None):
    with _pin_act_tables():
        return _build_inner(pre_loop_k, attn_loop_k, post_loop_k)


def _build_inner(pre_loop_k=None, attn_loop_k=None, post_loop_k=None):
    import contextlib

    nc = bacc.Bacc("TRN2", target_bir_lowering=False, debug=False, num_devices=NCORE)

    x_d = nc.dram_tensor("x", [C, N], f32, kind="ExternalInput")
    wq_d = nc.dram_tensor("wqTg", [C, 128], f32, kind="ExternalInput")
    wk_d = nc.dram_tensor("wkTg", [C, 128], f32, kind="ExternalInput")
    wv_d = nc.dram_tensor("wvTg", [C, D], f32, kind="ExternalInput")
    wo_d = nc.dram_tensor("woT", [C, C], f32, kind="ExternalInput")
    qbeta_d = nc.dram_tensor("qbeta", [128, 1], f32, kind="ExternalInput")
    kbeta_d = nc.dram_tensor("kbeta", [128, 1], f32, kind="ExternalInput")
    vbeta_d = nc.dram_tensor("vbeta", [1, D], f32, kind="ExternalInput")
    bo_d = nc.dram_tensor("bo", [C, 1], f32, kind="ExternalInput")
    resid_d = nc.dram_tensor("resid", [C, NT], f32, kind="ExternalInput")
    bones_d = nc.dram_tensor("bones", [128, 8], f32, kind="ExternalInput")
    emat_d = nc.dram_tensor("emat", [8, C], f32, kind="ExternalInput")
    out_d = nc.dram_tensor("out", [C, NT], f32, kind="ExternalOutput")

    with tile.TileContext(nc) as tc:
        with (
            tc.tile_pool(name="stg", bufs=1) as pstg,
            tc.tile_pool(name="xb", bufs=1) as pxb,
            tc.tile_pool(name="qk", bufs=1) as pqk,
            tc.tile_pool(name="w", bufs=1) as pw,
            tc.tile_pool(name="small", bufs=1) as psm,
            tc.tile_pool(name="p", bufs=3) as pp,
            tc.tile_pool(name="post", bufs=1) as ppost,
            tc.tile_pool(name="dram", bufs=1, space="DRAM") as pdram,
        ):
            pre_cm = (
                tc.For_i(0, pre_loop_k, 1) if pre_loop_k else contextlib.nullcontext()
            )
            pre_cm.__enter__()

            # ------------- stage 1: ingest x (bf16 convert + bn stats) --------
            xb = [pxb.tile([128, N], bf16, name=f"xb{i}") for i in range(CT)]
            stats = [psm.tile([128, 8, 6], f32, name=f"st{i}") for i in range(CT)]
            cstat = [psm.tile([128, 2], f32, name=f"cs{i}") for i in range(CT)]
            bones = psm.tile([128, 8], f32, name="bones")
            emat = psm.tile([8, C], f32, name="emat")
            nc.sync.dma_start(bones[:, :], bones_d[:, :])
            nc.sync.dma_start(emat[:, :], emat_d[:, :])

            # weights in early (off critical path)
            wq_sb = [pw.tile([128, 128], f32, name=f"wqf{i}") for i in range(CT)]
            wk_sb = [pw.tile([128, 128], f32, name=f"wkf{i}") for i in range(CT)]
            wv_sb = [pw.tile([128, D], f32, name=f"wvf{i}") for i in range(CT)]
            for i in range(CT):
                nc.sync.dma_start(wq_sb[i][:, :], wq_d[i * 128 : (i + 1) * 128, :])
                nc.sync.dma_start(wk_sb[i][:, :], wk_d[i * 128 : (i + 1) * 128, :])
                nc.sync.dma_start(wv_sb[i][:, :], wv_d[i * 128 : (i + 1) * 128, :])

            xc = [pstg.tile([128, N], f32, name=f"xc{i}") for i in range(CT)]
            msc = [psm.tile([128, 2], f32, name=f"msc{i}") for i in range(CT)]
            mscb = [psm.tile([128, 1], bf16, name=f"mscb{i}") for i in range(CT)]
            wqb = [pw.tile([128, 128], bf16, name=f"wqb{i}") for i in range(CT)]
            wkb = [pw.tile([128, 128], bf16, name=f"wkb{i}") for i in range(CT)]
            wvb = [pw.tile([128, D], bf16, name=f"wvb{i}") for i in range(CT)]
            eps_sb = psm.tile([8, 1], f32, name="eps_sb")
            nc.vector.memset(eps_sb[:, :], EPS)
            # DMAs first so the HW queue streams x continuously; halves so
            # bn_stats starts on the first half of each tile earlier
            for i in range(CT):
                for h in range(2):
                    nc.sync.dma_start(
                        xc[i][:, h * 2048 : (h + 1) * 2048],
                        x_d[i * 128 : (i + 1) * 128, h * 2048 : (h + 1) * 2048],
                    )
            with tc.tile_pool(name="ps_e", bufs=2, space="PSUM") as ps_e:
                for i in range(CT):
                    for u in range(NCHUNK):
                        nc.scalar.copy(
                            xb[i][:, u * 1024 : (u + 1) * 1024],
                            xc[i][:, u * 1024 : (u + 1) * 1024],
                        )
                        nc.vector.bn_stats(
                            out=stats[i][:, 2 * u, :],
                            in_=xc[i][:, u * 1024 : u * 1024 + 512],
                        )
                        nc.vector.bn_stats(
                            out=stats[i][:, 2 * u + 1, :],
                            in_=xc[i][:, u * 1024 + 512 : (u + 1) * 1024],
                        )
                    mv = psm.tile([128, 2], f32, name="mv", tag="mv", bufs=2)
                    nc.vector.bn_aggr(out=mv[:, :], in_=stats[i][:, :, :])
                    nc.vector.tensor_copy(cstat[i][:, 0:1], mv[:, 0:1])
                    nc.vector.tensor_mul(cstat[i][:, 1:2], mv[:, 0:1], mv[:, 0:1])
                    nc.vector.tensor_add(
                        cstat[i][:, 1:2], cstat[i][:, 1:2], mv[:, 1:2]
                    )
                    # group stats -> (mean, rstd) -> weight folds, immediately
                    # behind this tile's stats in every engine queue
                    gps = ps_e.tile([8, 2], f32, name="gps", tag="gps", bufs=2)
                    nc.tensor.matmul(gps[:, :], bones[:, :], cstat[i][:, :])
                    gm = psm.tile([8, 2], f32, name="gm", tag="gm", bufs=2)
                    nc.vector.tensor_scalar_mul(gm[:, :], gps[:, :], 1.0 / CPG)
                    varg = psm.tile([8, 1], f32, name="varg", tag="varg", bufs=2)
                    nc.vector.tensor_mul(varg[:, :], gm[:, 0:1], gm[:, 0:1])
                    nc.vector.tensor_sub(varg[:, :], gm[:, 1:2], varg[:, :])
                    lng = psm.tile([8, 1], f32, name="lng", tag="lng", bufs=2)
                    nc.scalar.activation(
                        lng[:, :], varg[:, :], AF.Ln, bias=eps_sb[:, :]
                    )
                    gsb = psm.tile([8, 2], f32, name="gsb", tag="gsb", bufs=2)
                    nc.vector.tensor_copy(gsb[:, 0:1], gm[:, 0:1])
                    nc.scalar.activation(gsb[:, 1:2], lng[:, :], AF.Exp, scale=-0.5)
                    msps = ps_e.tile([128, 2], f32, name="msps", tag="msps", bufs=2)
                    nc.tensor.matmul(
                        msps[:, :], emat[:, i * 128 : (i + 1) * 128], gsb[:, :]
                    )
                    nc.vector.tensor_copy(msc[i][:, :], msps[:, :])
                    nc.vector.tensor_copy(mscb[i][:, :], msc[i][:, 0:1])
                    nc.vector.tensor_scalar_mul(
                        wqb[i][:, :], wq_sb[i][:, :], msc[i][:, 1:2]
                    )
                    nc.vector.tensor_scalar_mul(
                        wkb[i][:, :], wk_sb[i][:, :], msc[i][:, 1:2]
                    )
                    nc.vector.tensor_scalar_mul(
                        wvb[i][:, :], wv_sb[i][:, :], msc[i][:, 1:2]
                    )

            # ------------- stage 3: q/k/v projections -------------------------
            kT2 = pqk.tile([128, N // 2], bf16, name="kT2")
            qT2 = pqk.tile([128, N], bf16, name="qT2")
            vaug = pqk.tile([128, NTK * VSTRIDE], bf16, name="vaug")
            nc.vector.memset(vaug[:, D :: VSTRIDE], 1.0)

            with (
                tc.tile_pool(name="ps_qk", bufs=2, space="PSUM") as ps_qk,
                tc.tile_pool(name="ps_v", bufs=2, space="PSUM") as ps_v,
            ):
                # k: i-outer accumulation across 4 live psum tiles so the PE
                # consumes each channel tile's matmuls as soon as its weight
                # fold lands (no head-of-line stall on the last tile's fold)
                kps4 = [
                    ps_qk.tile([128, 512], f32, name=f"kps{j}", bufs=1)
                    for j in range(4)
                ]
                for i in range(CT):
                    for j in range(4):
                        nc.tensor.matmul(
                            kps4[j][0:D, :],
                            wkb[i][:, 0:D],
                            xb[i][:, j * 512 : (j + 1) * 512],
                            start=(i == 0),
                            stop=(i == CT - 1),
                        )
                        nc.tensor.matmul(
                            kps4[j][D:128, :],
                            wkb[i][:, D:128],
                            xb[i][:, (j + 4) * 512 : (j + 5) * 512],
                            start=(i == 0),
                            stop=(i == CT - 1),
                        )

                # biases: cq = qbeta - wq' @ mean ; ck likewise ; cv as a row
                qbeta = psm.tile([128, 1], f32, name="qbeta")
                kbeta = psm.tile([128, 1], f32, name="kbeta")
                vbeta = psm.tile([1, D], f32, name="vbeta")
                nc.sync.dma_start(qbeta[:, :], qbeta_d[:, :])
                nc.sync.dma_start(kbeta[:, :], kbeta_d[:, :])
                nc.sync.dma_start(vbeta[:, :], vbeta_d[:, :])
                cq_sb = psm.tile([128, 1], f32, name="cq_sb")
                ck_sb = psm.tile([128, 1], f32, name="ck_sb")
                cv_row = psm.tile([1, D], f32, name="cv_row")
                bias_ps = ps_qk.tile([128, 512], f32, name="bias_ps", tag="qkps")
                for i in range(CT):
                    nc.tensor.matmul(
                        bias_ps[:, 0:1], wqb[i][:, :], mscb[i][:, :],
                        start=(i == 0), stop=(i == CT - 1),
                    )
                for i in range(CT):
                    nc.tensor.matmul(
                        bias_ps[:, 1:2], wkb[i][:, :], mscb[i][:, :],
                        start=(i == 0), stop=(i == CT - 1),
                    )
                for i in range(CT):
                    nc.tensor.matmul(
                        bias_ps[0:1, 2 : 2 + D], mscb[i][:, :], wvb[i][:, :],
                        start=(i == 0), stop=(i == CT - 1),
                    )
                nc.vector.tensor_sub(cq_sb[:, :], qbeta[:, :], bias_ps[:, 0:1])
                nc.vector.tensor_sub(ck_sb[:, :], kbeta[:, :], bias_ps[:, 1:2])
                nc.vector.tensor_sub(cv_row[:, :], vbeta[:, :], bias_ps[0:1, 2 : 2 + D])
                cvb = psm.tile([128, D], f32, name="cvb")
                nc.gpsimd.partition_broadcast(cvb[:, :], cv_row[:, :])


                # k evicts (need the ck bias, which lands after the last fold)
                for j in range(4):
                    nc.scalar.activation(
                        kT2[:, j * 512 : (j + 1) * 512],
                        kps4[j][:, :],
                        AF.Identity,
                        bias=ck_sb[:, :],
                    )
                # v: token-major tiles, bias-folded via broadcast add
                for jj in range(NTK):
                    vps = ps_v.tile([128, D], f32, name="vps", tag="vps")
                    for i in range(CT):
                        nc.tensor.matmul(
                            vps[:, :],
                            xb[i][:, jj * 128 : (jj + 1) * 128],
                            wvb[i][:, :],
                            start=(i == 0),
                            stop=(i == CT - 1),
                        )
                    slot = 2 * jj if jj < NPAIR else 2 * (jj - NPAIR) + 1
                    nc.vector.tensor_add(
                        vaug[:, slot * VSTRIDE : slot * VSTRIDE + D],
                        vps[:, :],
                        cvb[:, :],
                    )
                # q: duplicated into both partition halves via dup weights
                for j in range(NTQ):
                    qps = ps_qk.tile([128, 512], f32, name="qps", tag="qkps")
                    for i in range(CT):
                        nc.tensor.matmul(
                            qps[:, :],
                            wqb[i][:, :],
                            xb[i][:, j * 512 : (j + 1) * 512],
                            start=(i == 0),
                            stop=(i == CT - 1),
                        )
                    nc.scalar.activation(
                        qT2[:, j * 512 : (j + 1) * 512],
                        qps[:, :],
                        AF.Identity,
                        bias=cq_sb[:, :],
                    )

            pre_cm.__exit__(None, None, None)

            # ------------- stage 4: attention ---------------------------------
            a2a_in = pdram.tile([NTQ, D + 1, TQ], f8, name="a2a_in")
            sc65 = psm.tile([D + 1, 1], f32, name="sc65")
            nc.vector.memset(sc65[0:D, :], 1.0)
            nc.vector.memset(sc65[D : D + 1, :], DEN_SCALE)
            with (
                tc.tile_pool(name="ps_s", bufs=3, space="PSUM") as ps_s,
                tc.tile_pool(name="ps_o", bufs=1, space="PSUM") as ps_o,
            ):
                loop_cm = (
                    tc.For_i(
                        0,
                        attn_loop_k,
                        1,
                        hint_engines=(
                            mybir.EngineType.PE,
                            mybir.EngineType.Activation,
                        ),
                    )
                    if attn_loop_k
                    else contextlib.nullcontext()
                )
                with loop_cm:
                    for jq in range(NTQ):
                        _attention_jq(
                            nc, tc, jq, ps_s, ps_o, pp, kT2, qT2, vaug, a2a_in,
                            sc65,
                        )

            # ------------- stage 5: AllToAll ----------------------------------
            a2a_out = pdram.tile([NTQ, D + 1, TQ], f8, name="a2a_out")
            nc.gpsimd.collective_compute(
                "AllToAll",
                ALU.bypass,
                replica_groups=[list(range(NCORE))],
                ins=[a2a_in.opt()],
                outs=[a2a_out.opt()],
            )

            # ------------- stage 6: normalize + output projection -------------
            # prep during attention: wo bf16, resid + bo
            wo_sb = [ppost.tile([128, C], f32, name=f"wo{i}") for i in range(CT)]
            wob = [ppost.tile([128, C], bf16, name=f"wob{i}") for i in range(CT)]
            resid_sb = [ppost.tile([128, NT], f32, name=f"res{i}") for i in range(CT)]
            resid3 = [ppost.tile([128, NT], f32, name=f"res3{i}") for i in range(CT)]
            bo_sb = ppost.tile([128, CT], f32, name="bo_sb")
            for i in range(CT):
                nc.sync.dma_start(wo_sb[i][:, :], wo_d[i * 128 : (i + 1) * 128, :])
                nc.vector.tensor_copy(wob[i][:, :], wo_sb[i][:, :])
                nc.sync.dma_start(
                    resid_sb[i][:, :], resid_d[i * 128 : (i + 1) * 128, :]
                )
                nc.sync.dma_start(bo_sb[:, i : i + 1], bo_d[i * 128 : (i + 1) * 128, :])
                nc.vector.tensor_scalar(
                    out=resid3[i][:, :],
                    in0=resid_sb[i][:, :],
                    scalar1=bo_sb[:, i : i + 1],
                    scalar2=None,
                    op0=ALU.add,
                )

            post_cm = (
                tc.For_i(0, post_loop_k, 1) if post_loop_k else contextlib.nullcontext()
            )
            post_cm.__enter__()

            rhs_raw = ppost.tile([128, 4 * TQ], bf16, name="rhs_raw")
            denb = ppost.tile([128, 4 * TQ], bf16, name="denb")
            src_pv = a2a_out[:, 0:D, :].rearrange("(i j) d t -> j d i t", j=2)
            src_rc = a2a_out[:, D : D + 1, :].rearrange("(i j) d t -> j d i t", j=2)
            for j in range(2):
                nc.gpsimd.dma_start(
                    denb[j * D : (j + 1) * D, :].rearrange("d (i t) -> d i t", i=4),
                    src_rc[j].broadcast_to([D, 4, TQ]),
                )
            for j in range(2):
                nc.gpsimd.dma_start(
                    rhs_raw[j * D : (j + 1) * D, :].rearrange(
                        "d (i t) -> d i t", i=4
                    ),
                    src_pv[j],
                )
            dlnb = ppost.tile([128, 4 * TQ], f32, name="dlnb")
            nc.scalar.activation(dlnb[:, :], denb[:, :], AF.Ln)
            lnsc = ppost.tile([128, 1], f32, name="lnsc")
            nc.vector.memset(lnsc[:, :], LN_DEN_SCALE)
            rcpb = ppost.tile([128, 4 * TQ], bf16, name="rcpb")
            nc.scalar.activation(
                rcpb[:, :], dlnb[:, :], AF.Exp, scale=-1.0, bias=lnsc[:, :]
            )
            rhs_n = ppost.tile([128, 4 * TQ], bf16, name="rhs_n")
            nc.vector.tensor_mul(rhs_n[:, :], rhs_raw[:, :], rcpb[:, :])

            with tc.tile_pool(name="ps_y", bufs=2, space="PSUM") as ps_y:
                for c in range(CT):
                    yps = ps_y.tile([128, NT], f32, name="yps", tag="yps")
                    for i in range(CT):
                        nc.tensor.matmul(
                            yps[:, :],
                            wob[i][:, c * 128 : (c + 1) * 128],
                            rhs_n[:, i * TQ : (i + 1) * TQ],
                            start=(i == 0),
                            stop=(i == CT - 1),
                        )
                    y_sb = ppost.tile([128, NT], f32, name="y_sb", tag="y_sb", bufs=2)
                    nc.vector.tensor_add(y_sb[:, :], yps[:, :], resid3[c][:, :])
                    nc.sync.dma_start(out_d[c * 128 : (c + 1) * 128, :], y_sb[:, :])
            post_cm.__exit__(None, None, None)

    nc.compile()
    return nc


def get_nc():
    if "nc" not in _nc_cache:
        _nc_cache["nc"] = _build()
    return _nc_cache["nc"]


def make_in_maps(hidden_states, gn_gamma, gn_beta, wq, wk, wv, wo, bo):
    x2d = np.ascontiguousarray(
        np.asarray(hidden_states, dtype=np.float32).reshape(C, N)
    )
    gamma = np.asarray(gn_gamma, np.float32)
    beta = np.asarray(gn_beta, np.float32)
    wq = np.asarray(wq, np.float32)
    wk = np.asarray(wk, np.float32)
    wv = np.asarray(wv, np.float32)
    woT = np.ascontiguousarray(np.asarray(wo, np.float32).T)
    bo2 = np.ascontiguousarray(np.asarray(bo, np.float32).reshape(C, 1))

    # per-tile group-sum matrix and group->channel expansion matrix
    bones = np.zeros((128, 8), np.float32)
    emat = np.zeros((8, C), np.float32)
    for cc in range(128):
        bones[cc, cc // CPG] = 1.0
    for i in range(CT):
        for cc in range(128):
            emat[cc // CPG, i * 128 + cc] = 1.0

    in_maps = []
    for h in range(NCORE):
        sl = slice(h * D, (h + 1) * D)
        wqg = np.ascontiguousarray((wq[sl, :] * gamma[None, :]).T)  # [C, 64]
        wkg = np.ascontiguousarray((wk[sl, :] * gamma[None, :]).T)
        wvg = np.ascontiguousarray((wv[sl, :] * gamma[None, :]).T)
        qb = (wq[sl, :] @ beta).astype(np.float32)  # [64]
        kb = (wk[sl, :] @ beta).astype(np.float32)
        vb = (wv[sl, :] @ beta).astype(np.float32)
        in_maps.append(
            {
                "x": x2d,
                "wqTg": np.ascontiguousarray(np.concatenate([wqg, wqg], axis=1)),
                "wkTg": np.ascontiguousarray(np.concatenate([wkg, wkg], axis=1)),
                "wvTg": wvg,
                "woT": woT,
                "qbeta": np.ascontiguousarray(
                    np.concatenate([qb, qb]).reshape(128, 1)
                ),
                "kbeta": np.ascontiguousarray(
                    np.concatenate([kb, kb]).reshape(128, 1)
                ),
                "vbeta": np.ascontiguousarray(vb.reshape(1, D)),
                "bo": bo2,
                "resid": np.ascontiguousarray(x2d[:, h * NT : (h + 1) * NT]),
                "bones": bones,
                "emat": emat,
            }
        )
    return in_maps


def kernel(hidden_states, gn_gamma, gn_beta, wq, wk, wv, wo, bo):
    nc = get_nc()
    in_maps = make_in_maps(hidden_states, gn_gamma, gn_beta, wq, wk, wv, wo, bo)
    res = run_bass_kernel_spmd(nc, in_maps, core_ids=list(range(NCORE)))
    out2d = np.empty((C, N), np.float32)
    for h in range(NCORE):
        out2d[:, h * NT : (h + 1) * NT] = res.results[h]["out"]
    return out2d.reshape(1, C, 64, 64)
